# revision 8
# baseline (speedup 1.0000x reference)
"""Causal self-attention with RoPE for Trainium2, 8 NeuronCores.

Sharding: core c = (batch b = c//2, head-group g = c%2 of 8 heads).
Each core computes qkv for its 8 heads, RoPE, causal attention (prefix
masks derived from sorted `indices`), and a partial c_proj (its 512
input channels). Host sums the two partial c_proj outputs per batch.

Device algorithm:
  - qkv + c_proj matmuls in float32r (TF32-class); attention operand
    tensors (qT/kT/v/E/masks) in bf16 (measured end-to-end ~2.5e-3).
  - attention computed directly in transposed orientation E^T[k, q], so
    softmax needs no transposes: denominators come from a ones column
    appended to V (row 64 of the M=65 y matmul), and normalization is a
    per-query scale applied before c_proj (fused into the PSUM->SBUF
    multiply by 1/S broadcast across partitions).
  - causal masking via a -1e9 bias accumulated into the logits PSUM by
    an extra matmul: diag(-1e9) (stationary) x 0/1 prefix mask tile
    built on-device with one tensor_scalar (count <= iota) op.
  - odd heads' normalized y tiles are shifted to partitions 64-127 via
    SBUF->SBUF DMA so c_proj runs K=128 matmuls.
"""

import numpy as np

B, T, C, H = 4, 2048, 1024, 16
HD = 64
HC = 8            # heads per core
NCORES = 8
PT = 128          # partition tile
TT = T // PT      # 16 T-tiles
QCW = 512         # q-chunk width
NQC = T // QCW    # 4
NKT = T // PT     # 16 key tiles
VW = HD + 1       # 65: v columns + ones column

_PROG_CACHE: dict = {}
_last_in_maps = None


def _build_program(kmax, boundary):
    import concourse.bass as bass
    import concourse.tile as tile
    from concourse import bacc, mybir
    from concourse.masks import make_identity

    F32 = mybir.dt.float32
    F32R = mybir.dt.float32r
    BF16 = mybir.dt.bfloat16
    AT = mybir.ActivationFunctionType
    OP = mybir.AluOpType

    nc = bacc.Bacc("TRN2", target_bir_lowering=False, debug=False)

    x_d = nc.dram_tensor("x", [T, C], F32, kind="ExternalInput")
    wqkv_d = nc.dram_tensor("wqkv", [C, 3 * QCW], F32R, kind="ExternalInput")
    wp_d = nc.dram_tensor("wp", [QCW, C], F32R, kind="ExternalInput")
    cos_d = nc.dram_tensor("cosn", [T, 32], F32, kind="ExternalInput")
    sin_d = nc.dram_tensor("sinn", [T, 32], F32, kind="ExternalInput")
    cntb_d = nc.dram_tensor("cntb", [PT, T], F32, kind="ExternalInput")
    iota_d = nc.dram_tensor("iotas", [PT, NKT], F32, kind="ExternalInput")
    out_d = nc.dram_tensor("out", [T, C], F32, kind="ExternalOutput")

    with tile.TileContext(nc) as tc:
        with (
            tc.tile_pool(name="persist", bufs=1) as pp,
            tc.tile_pool(name="wq", bufs=1) as wqp,
            tc.tile_pool(name="work", bufs=2) as wk,
            tc.tile_pool(name="epool", bufs=4) as ep,
            tc.tile_pool(name="bpool", bufs=6) as bp,
            tc.tile_pool(name="ipool", bufs=2) as ip,
            tc.tile_pool(name="osb", bufs=2) as op_,
            tc.tile_pool(name="psA", bufs=3, space="PSUM") as psA,
            tc.tile_pool(name="psE", bufs=3, space="PSUM") as psE,
            tc.tile_pool(name="psY", bufs=2, space="PSUM") as psY,
        ):
            # ---------------- persistent tiles ----------------
            qT = [pp.tile([PT, T], BF16, tag=f"qT{g}", name=f"qT{g}") for g in range(4)]
            kT = [pp.tile([PT, T], BF16, tag=f"kT{g}", name=f"kT{g}") for g in range(4)]
            yTp = [pp.tile([PT, T], F32R, tag=f"yTp{g}", name=f"yTp{g}") for g in range(4)]
            vaug = pp.tile([PT, NKT * HC * VW], BF16, tag="vaug")
            cos_sb = pp.tile([PT, TT * 32], F32, tag="cos")
            sin_sb = pp.tile([PT, TT * 32], F32, tag="sin")
            cntb = pp.tile([PT, T], F32, tag="cntb")
            iotas = pp.tile([PT, NKT], F32, tag="iotas")
            ident = pp.tile([PT, PT], F32, tag="ident")
            negid = pp.tile([PT, PT], BF16, tag="negid")
            ones = pp.tile([PT, 1], F32, tag="ones")

            w_sb = wqp.tile([PT, (C // PT) * 3 * QCW], F32R, tag="w")

            nc.sync.dma_start(cos_sb[:].rearrange("p (t j) -> p t j", j=32),
                              cos_d[:].rearrange("(t p) j -> p t j", p=PT))
            nc.sync.dma_start(sin_sb[:].rearrange("p (t j) -> p t j", j=32),
                              sin_d[:].rearrange("(t p) j -> p t j", p=PT))
            nc.sync.dma_start(cntb[:], cntb_d[:])
            nc.sync.dma_start(iotas[:], iota_d[:])
            nc.sync.dma_start(w_sb[:].rearrange("p (k n) -> p k n", n=3 * QCW),
                              wqkv_d[:].rearrange("(k p) n -> p k n", p=PT))
            make_identity(nc, ident[:])
            nc.scalar.mul(negid[:], ident[:], -1e9)
            nc.vector.memset(ones[:], 1.0)
            ones_ap = ones[:]
            ones_rep = bass.AP(ones_ap.tensor, ones_ap.offset,
                               [ones_ap.ap[0], [0, NKT], [0, HC]])
            nc.vector.tensor_copy(
                vaug[:].rearrange("p (t h c) -> p t h c", h=HC, c=VW)[:, :, :, HD],
                ones_rep,
            )

            # ---------------- phase 1: qkv + rope + transposes ----------------
            for t in range(TT):
                x_t = wk.tile([PT, C], F32, tag="x")
                nc.sync.dma_start(x_t[:], x_d[t * PT:(t + 1) * PT, :])

                xT_t = wk.tile([PT, C], F32R, tag="xT")
                for c in range(C // PT):
                    tp = psA.tile([PT, PT], F32, tag="mm")
                    nc.tensor.transpose(tp[:], x_t[:, c * PT:(c + 1) * PT], ident[:])
                    nc.scalar.copy(xT_t[:, c * PT:(c + 1) * PT], tp[:])

                stag = wk.tile([PT, 2 * QCW], F32, tag="stag")
                for ch in range(3):  # q, k, v
                    ps = psA.tile([PT, QCW], F32, tag="mm")
                    for c in range(C // PT):
                        nc.tensor.matmul(
                            ps[:],
                            xT_t[:, c * PT:(c + 1) * PT],
                            w_sb[:, c * 3 * QCW + ch * QCW: c * 3 * QCW + (ch + 1) * QCW],
                            start=(c == 0), stop=(c == C // PT - 1),
                        )
                    if ch == 2:  # v -> vaug (strided dest, skip ones cols)
                        nc.vector.tensor_copy(
                            vaug[:, t * HC * VW:(t + 1) * HC * VW]
                            .rearrange("p (h c) -> p h c", c=VW)[:, :, 0:HD],
                            ps[:].rearrange("p (h c) -> p h c", c=HD),
                        )
                    else:
                        nc.vector.tensor_copy(
                            stag[:, ch * QCW:(ch + 1) * QCW], ps[:]
                        )

                # rope on q|k staging -> rot
                rot = wk.tile([PT, 2 * QCW], F32, tag="rot")
                sv = stag[:].rearrange("p (g two j) -> p g two j", two=2, j=32)
                rv = rot[:].rearrange("p (g two j) -> p g two j", two=2, j=32)
                X1, X2 = sv[:, :, 0, :], sv[:, :, 1, :]
                R1, R2 = rv[:, :, 0, :], rv[:, :, 1, :]
                cos_ap = cos_sb[:, t * 32:(t + 1) * 32]
                sin_ap = sin_sb[:, t * 32:(t + 1) * 32]
                cosr = bass.AP(cos_ap.tensor, cos_ap.offset,
                               [cos_ap.ap[0], [0, 16], [1, 32]])
                sinr = bass.AP(sin_ap.tensor, sin_ap.offset,
                               [sin_ap.ap[0], [0, 16], [1, 32]])
                t1 = wk.tile([PT, QCW], F32, tag="tmp1", bufs=1)
                t2 = wk.tile([PT, QCW], F32, tag="tmp2", bufs=1)
                t1v = t1[:].rearrange("p (g j) -> p g j", j=32)
                t2v = t2[:].rearrange("p (g j) -> p g j", j=32)
                nc.vector.tensor_tensor(t1v, X1, cosr, OP.mult)
                nc.vector.tensor_tensor(t2v, X2, sinr, OP.mult)
                nc.vector.tensor_tensor(R1, t1v, t2v, OP.subtract)
                t3 = wk.tile([PT, QCW], F32, tag="tmp1", bufs=1)
                t4 = wk.tile([PT, QCW], F32, tag="tmp2", bufs=1)
                t3v = t3[:].rearrange("p (g j) -> p g j", j=32)
                t4v = t4[:].rearrange("p (g j) -> p g j", j=32)
                nc.vector.tensor_tensor(t3v, X1, sinr, OP.mult)
                nc.vector.tensor_tensor(t4v, X2, cosr, OP.mult)
                nc.vector.tensor_tensor(R2, t3v, t4v, OP.add)

                for c in range(8):
                    tp = psA.tile([PT, PT], F32, tag="mm")
                    nc.tensor.transpose(tp[:], rot[:, c * PT:(c + 1) * PT], ident[:])
                    dst = qT[c] if c < 4 else kT[c - 4]
                    nc.scalar.copy(dst[:, t * PT:(t + 1) * PT], tp[:])

            # ---------------- phase 2: attention ----------------
            for J in range(NQC):
                qs = slice(J * QCW, (J + 1) * QCW)
                bts = {}
                for i in range(kmax[J]):
                    if (i, J) in boundary:
                        bt = bp.tile([PT, QCW], BF16, tag="B")
                        nc.vector.tensor_scalar(
                            bt[:], cntb[:, qs], iotas[:, i:i + 1], None, OP.is_le
                        )
                        bts[i] = bt
                for g in range(4):
                    yA = psY.tile([VW, QCW], F32, tag="y")
                    yB = psY.tile([VW, QCW], F32, tag="y")
                    last = kmax[J] - 1
                    for i in range(kmax[J]):
                        ks = slice(i * PT, (i + 1) * PT)
                        bnd = i in bts
                        for hh, ybank in ((0, yA), (1, yB)):
                            base = 64 * hh
                            et = psE.tile([PT, QCW], F32, tag="et")
                            nc.tensor.matmul(
                                et[:],
                                kT[g][base:base + HD, ks],
                                qT[g][base:base + HD, qs],
                                start=True, stop=not bnd,
                            )
                            if bnd:
                                nc.tensor.matmul(
                                    et[:], negid[:], bts[i][:],
                                    start=False, stop=True,
                                )
                            e_sb = ep.tile([PT, QCW], BF16, tag="E")
                            nc.scalar.activation(e_sb[:], et[:], AT.Exp, scale=0.125)
                            h = 2 * g + hh
                            vcol = i * HC * VW + h * VW
                            nc.tensor.matmul(
                                ybank[:], vaug[:, vcol:vcol + VW], e_sb[:],
                                start=(i == 0), stop=(i == last),
                                skip_group_check=True,
                            )
                    # drain: S rows -> recip -> broadcast -> fused normalize
                    st = wk.tile([PT, 2 * QCW], F32, tag="st", bufs=1)
                    nc.vector.reciprocal(st[64:65, 0:QCW], yA[HD:VW, :])
                    nc.vector.reciprocal(st[64:65, QCW:2 * QCW], yB[HD:VW, :])
                    nc.sync.dma_start(st[0:1, :], st[64:65, :])
                    invcA = ip.tile([64, QCW], F32, tag="invc")
                    invcB = ip.tile([64, QCW], F32, tag="invc")
                    nc.gpsimd.partition_broadcast(invcA[:], st[0:1, 0:QCW])
                    nc.gpsimd.partition_broadcast(invcB[:], st[0:1, QCW:2 * QCW])
                    nc.vector.tensor_tensor(
                        yTp[g][0:HD, qs], yA[0:HD, :], invcA[:], OP.mult
                    )
                    ytmp = wk.tile([64, QCW], F32R, tag="ytmp")
                    nc.vector.tensor_tensor(
                        ytmp[:], yB[0:HD, :], invcB[:], OP.mult
                    )
                    nc.sync.dma_start(yTp[g][HD:2 * HD, qs], ytmp[:])

            # ---------------- phase 3: c_proj ----------------
            wp_sb = wqp.tile([PT, (QCW // PT) * C], F32R, tag="w")
            nc.sync.dma_start(wp_sb[:].rearrange("p (k n) -> p k n", n=C),
                              wp_d[:].rearrange("(k p) n -> p k n", p=PT))
            for t in range(TT):
                for n in range(C // QCW):
                    ps = psA.tile([PT, QCW], F32, tag="mm")
                    for k4 in range(QCW // PT):
                        nc.tensor.matmul(
                            ps[:],
                            yTp[k4][:, t * PT:(t + 1) * PT],
                            wp_sb[:, k4 * C + n * QCW: k4 * C + (n + 1) * QCW],
                            start=(k4 == 0), stop=(k4 == QCW // PT - 1),
                        )
                    o_sb = op_.tile([PT, QCW], F32, tag="osb")
                    nc.scalar.copy(o_sb[:], ps[:])
                    nc.sync.dma_start(
                        out_d[t * PT:(t + 1) * PT, n * QCW:(n + 1) * QCW], o_sb[:]
                    )

    nc.compile()
    return nc


def _get_program(kmax, boundary):
    key = (tuple(kmax), frozenset(boundary))
    if key not in _PROG_CACHE:
        _PROG_CACHE[key] = _build_program(list(kmax), set(boundary))
    return _PROG_CACHE[key]


def _prep(x, W_attn, W_proj, indices):
    half = HD // 2
    inv_freq = (1.0 / (10000.0 ** (np.arange(half, dtype=np.float32)
                                   / np.float32(half)))).astype(np.float32)

    counts = np.empty((B, T), np.int64)
    for b in range(B):
        counts[b] = np.searchsorted(indices[b], indices[b], side="right")

    cmax = counts.max(axis=0)
    cmin = counts.min(axis=0)
    kmax = []
    boundary = set()
    for J in range(NQC):
        hi = int(cmax[J * QCW:(J + 1) * QCW].max())
        lo = int(cmin[J * QCW:(J + 1) * QCW].min())
        km = (hi + PT - 1) // PT
        kmax.append(km)
        for i in range(km):
            if not (lo >= (i + 1) * PT):
                boundary.add((i, J))

    iotas = (np.arange(PT, dtype=np.float32)[:, None]
             + PT * np.arange(NKT, dtype=np.float32)[None, :]).copy()

    in_maps = []
    for core in range(NCORES):
        b, g = core // 2, core % 2
        wq = W_attn[:, g * QCW:(g + 1) * QCW]
        wk_ = W_attn[:, C + g * QCW: C + (g + 1) * QCW]
        wv = W_attn[:, 2 * C + g * QCW: 2 * C + (g + 1) * QCW]
        wqkv = np.ascontiguousarray(np.concatenate([wq, wk_, wv], axis=1))
        wp = np.ascontiguousarray(W_proj[g * QCW:(g + 1) * QCW, :])
        ang = indices[b].astype(np.float32)[:, None] * inv_freq[None, :]
        in_maps.append({
            "x": np.ascontiguousarray(x[b]),
            "wqkv": wqkv,
            "wp": wp,
            "cosn": np.cos(ang).astype(np.float32),
            "sinn": np.sin(ang).astype(np.float32),
            "cntb": np.broadcast_to(
                counts[b].astype(np.float32)[None, :], (PT, T)).copy(),
            "iotas": iotas,
        })
    return kmax, boundary, in_maps


def kernel(x, W_attn, W_proj, indices):
    global _last_in_maps
    x = np.asarray(x, dtype=np.float32)
    W_attn = np.asarray(W_attn, dtype=np.float32)
    W_proj = np.asarray(W_proj, dtype=np.float32)
    indices = np.asarray(indices)

    kmax, boundary, in_maps = _prep(x, W_attn, W_proj, indices)
    _last_in_maps = in_maps
    nc = _get_program(kmax, boundary)

    from concourse.bass_utils import run_bass_kernel_spmd
    res = run_bass_kernel_spmd(nc, in_maps, list(range(NCORES)))

    out = np.empty((B, T, C), np.float32)
    for b in range(B):
        out[b] = res.results[2 * b]["out"] + res.results[2 * b + 1]["out"]
    return out
